# revision 7
# baseline (speedup 1.0000x reference)
"""AdaptConv2d Trainium2 kernel: per-sample adapted 1x1 conv (SE-modulated).

Reference computation (B=32, C=O=256, H=W=56, HID=16):
    pooled = mean(x, (2,3))                      [B, C]
    hid    = silu(pooled @ w_se1.T)              [B, 16]
    adapt  = (hid @ w_se_out.T).reshape(B,O,C)   [B, O, C]
    out[b] = (w_conv + adapt[b]) @ x[b]          [B, O, H*W]

Distribution: data-parallel over batch, 4 samples per core on 8 cores, no
collectives. Per core everything is done on-chip:
  - x streamed in as bf16 (host pre-cast), kept SBUF-resident
  - pooling via tensor_tensor_reduce (free-dim sum of the two halves plus
    accumulate) while x loads
  - hid: f32 matmul contracting C on the PE, SiLU on ACT
  - adapt: single bf16 PE pass over w_se_out with a block-diagonal hid
    stationary operand (8 groups of 16 h-channels packed into the 128
    contraction rows -> 8x fewer streaming columns)
  - adapt is evicted to SBUF and reshaped into per-sample lhsT layout
    [C, O] via SBUF->SBUF DMAs, then w_conv is added
  - main GEMM: per sample [O,C]@[C,HW] as bf16 matmuls accumulating over
    C in PSUM, evicted to bf16 staging, batched 0.8MB output DMAs
"""

import numpy as np

B_PER_CORE = 4
N_CORES = 8
C = 256
O = 256
HW = 3136
HID = 16
P = 128
NT = 448  # free-dim tile of the main GEMM (7 * 448 = 3136)
NN = HW // NT
G = 8  # h-group packing factor for the adapt matmul
SEG = C * O // G // 256  # 32 c-rows per group block
ADN = C * O // G  # 8192 streaming columns for adapt
ADT = ADN // 512  # 16 psum tiles


def build_nc():
    from concourse import bacc, tile, mybir

    f32 = mybir.dt.float32
    bf16 = mybir.dt.bfloat16

    nc = bacc.Bacc("TRN2", target_bir_lowering=False, debug=False)

    x_d = nc.dram_tensor("x", [B_PER_CORE, P, 2 * HW], bf16, kind="ExternalInput")
    wse_d = nc.dram_tensor("w_se", [P, ADN], bf16, kind="ExternalInput")
    wconv_d = nc.dram_tensor("w_conv", [P, 2 * O], bf16, kind="ExternalInput")
    wse1_d = nc.dram_tensor("w_se1", [P, 2 * HID], f32, kind="ExternalInput")
    out_d = nc.dram_tensor(
        "out", [B_PER_CORE, 2, P, HW], bf16, kind="ExternalOutput"
    )

    with tile.TileContext(nc) as tc:
        with (
            tc.tile_pool(name="xp", bufs=B_PER_CORE) as xp,
            tc.tile_pool(name="wsep", bufs=1) as wsep,
            tc.tile_pool(name="consts", bufs=1) as consts,
            tc.tile_pool(name="scratch", bufs=2) as scratchp,
            tc.tile_pool(name="small", bufs=1) as small,
            tc.tile_pool(name="stage", bufs=2) as stagep,
            tc.tile_pool(name="ps_hid", bufs=1, space="PSUM") as ps_hid,
            tc.tile_pool(name="ps_ad", bufs=2, space="PSUM") as ps_ad,
            tc.tile_pool(name="ps_mm", bufs=3, space="PSUM") as ps_mm,
        ):
            # ---- load phase ----
            x_tiles = []
            for b in range(B_PER_CORE):
                xt = xp.tile([P, 2 * HW], bf16, tag="x")
                nc.sync.dma_start(out=xt[:], in_=x_d.ap()[b])
                x_tiles.append(xt)
            wse_sb = wsep.tile([P, ADN], bf16)
            nc.sync.dma_start(out=wse_sb[:], in_=wse_d.ap()[:])
            wconv_sb = consts.tile([P, 2 * O], bf16, tag="wconv")
            nc.sync.dma_start(out=wconv_sb[:], in_=wconv_d.ap()[:])
            wse1_sb = consts.tile([P, 2 * HID], f32, tag="wse1")
            nc.sync.dma_start(out=wse1_sb[:], in_=wse1_d.ap()[:])

            # ---- pooling (gpsimd; runs while later x tiles stream in) ----
            # pooled_k[:, b] = sum over hw of x[b, c_chunk k]
            pooled = [
                small.tile([P, B_PER_CORE], f32, tag=f"pooled{k}", name=f"pooled{k}")
                for k in range(2)
            ]
            half = HW // 2
            for b in range(B_PER_CORE):
                for k in range(2):
                    if k == 0:
                        nc.vector.tensor_reduce(
                            out=pooled[k][:, b : b + 1],
                            in_=x_tiles[b][:, k * HW : (k + 1) * HW],
                            axis=mybir.AxisListType.X,
                            op=mybir.AluOpType.add,
                        )
                    else:
                        sca = scratchp.tile([P, HW], bf16, tag="poolscratch_a")
                        nc.scalar.activation(
                            out=sca[:],
                            in_=x_tiles[b][:, k * HW : (k + 1) * HW],
                            func=mybir.ActivationFunctionType.Copy,
                            accum_out=pooled[k][:, b : b + 1],
                        )

            # ---- hid = silu(pooled @ w_se1.T) as [16, 4] ----
            hid_ps = ps_hid.tile([HID, B_PER_CORE], f32)
            for k in range(2):
                nc.tensor.matmul(
                    hid_ps[:],
                    lhsT=wse1_sb[:, k * HID : (k + 1) * HID],
                    rhs=pooled[k][:],
                    start=(k == 0),
                    stop=(k == 1),
                )
            hid_sb = small.tile([HID, B_PER_CORE], bf16, tag="hid")
            sig_sb = small.tile([HID, B_PER_CORE], f32, tag="sig")
            nc.scalar.activation(
                sig_sb[:], hid_ps[:], mybir.ActivationFunctionType.Sigmoid
            )
            nc.vector.tensor_tensor(
                out=hid_sb[:], in0=sig_sb[:], in1=hid_ps[:], op=mybir.AluOpType.mult
            )

            # ---- block-diagonal stationary operand for the adapt matmul ----
            bd = small.tile([P, G * B_PER_CORE], bf16, tag="bd")
            nc.vector.memset(bd[:], 0.0)
            for g in range(G):
                nc.scalar.dma_start(
                    out=bd[g * HID : (g + 1) * HID, g * B_PER_CORE : (g + 1) * B_PER_CORE],
                    in_=hid_sb[:],
                )

            # ---- adapt matmul: out[(g,b), n] = adapt[b, flat=g*8192+n] ----
            # (flat index = c*256 + o)
            adapt_sb = wsep.tile([G * B_PER_CORE, ADN], bf16, tag="adapt")
            for t in range(ADT):
                ap_ps = ps_ad.tile([G * B_PER_CORE, 512], f32, tag="adps")
                nc.tensor.matmul(
                    ap_ps[:],
                    lhsT=bd[:],
                    rhs=wse_sb[:, t * 512 : (t + 1) * 512],
                    start=True,
                    stop=True,
                )
                nc.vector.tensor_copy(
                    out=adapt_sb[:, t * 512 : (t + 1) * 512], in_=ap_ps[:]
                )

            # ---- reshape adapt rows into lhsT layout + add w_conv ----
            # wb_raw[:, (b*2+k)*256 + o] over partitions c_local = lhsT[c, o]
            wb_raw = consts.tile([P, B_PER_CORE * 2 * O], bf16, tag="wbraw")
            wb = consts.tile([P, B_PER_CORE * 2 * O], bf16, tag="wb")
            for b in range(B_PER_CORE):
                for k in range(2):
                    col0 = (b * 2 + k) * O
                    for gl in range(4):
                        g = k * 4 + gl
                        r = g * B_PER_CORE + b
                        nc.scalar.dma_start(
                            out=wb_raw[gl * SEG : (gl + 1) * SEG, col0 : col0 + O],
                            in_=adapt_sb[r : r + 1, :],
                        )
                    nc.vector.tensor_tensor(
                        out=wb[:, col0 : col0 + O],
                        in0=wb_raw[:, col0 : col0 + O],
                        in1=wconv_sb[:, k * O : (k + 1) * O],
                        op=mybir.AluOpType.add,
                    )

            # ---- main GEMM ----
            for b in range(B_PER_CORE):
                for oc in range(2):
                    stage = stagep.tile([P, HW], bf16, tag="stage")
                    for n in range(NN):
                        ps = ps_mm.tile([P, NT], f32, tag="mmps")
                        for k in range(2):
                            nc.tensor.matmul(
                                ps[:],
                                lhsT=wb[:, (b * 2 + k) * O + oc * P : (b * 2 + k) * O + oc * P + P],
                                rhs=x_tiles[b][:, k * HW + n * NT : k * HW + (n + 1) * NT],
                                start=(k == 0),
                                stop=(k == 1),
                            )
                        if n % 2 == 0:
                            nc.vector.tensor_copy(
                                out=stage[:, n * NT : (n + 1) * NT], in_=ps[:]
                            )
                        else:
                            nc.scalar.copy(
                                out=stage[:, n * NT : (n + 1) * NT], in_=ps[:]
                            )
                    nc.scalar.dma_start(out=out_d.ap()[b, oc], in_=stage[:])

    nc.compile()
    return nc


def prep_core_inputs(x_shard, w_conv, w_se1, w_se_out):
    """Host-side layout prep for one core. x_shard: [4, 256, 56, 56] f32."""
    import ml_dtypes

    bf16 = ml_dtypes.bfloat16
    b = x_shard.shape[0]
    # x: [b, 128, 2*3136], c = k*128 + p, free = k*3136 + hw
    xr = x_shard.reshape(b, 2, P, HW).transpose(0, 2, 1, 3).reshape(b, P, 2 * HW)
    x_dev = np.ascontiguousarray(xr).astype(bf16)
    # w_se: [(g,h), n] with flat = c*256 + o = g*8192 + n
    w_r = w_se_out.reshape(O, C, HID).transpose(1, 0, 2)  # [c, o, h]
    w_r = w_r.reshape(G, ADN, HID).transpose(0, 2, 1).reshape(P, ADN)
    wse_dev = np.ascontiguousarray(w_r).astype(bf16)
    # w_conv: [p, k*256 + o] = w_conv[o, k*128+p]
    wc = w_conv[:, :, 0, 0].T.reshape(2, P, O).transpose(1, 0, 2).reshape(P, 2 * O)
    wconv_dev = np.ascontiguousarray(wc).astype(bf16)
    # w_se1: [p, k*16 + h] = w_se1[h, k*128+p] / 3136  (fold the mean divisor)
    w1 = (w_se1.T / float(HW)).reshape(2, P, HID).transpose(1, 0, 2).reshape(P, 2 * HID)
    wse1_dev = np.ascontiguousarray(w1).astype(np.float32)
    return {"x": x_dev, "w_se": wse_dev, "w_conv": wconv_dev, "w_se1": wse1_dev}


def postprocess(raw_out):
    """raw_out: [4, 2, 128, 3136] bf16 -> [4, 256, 56, 56] f32."""
    return np.asarray(raw_out, dtype=np.float32).reshape(B_PER_CORE, O, 56, 56)


_NC_CACHE = None
LAST_RESULT = None


def kernel(x, w_conv, w_se1, w_se_out):
    global _NC_CACHE
    from concourse.bass_utils import run_bass_kernel_spmd

    if _NC_CACHE is None:
        _NC_CACHE = build_nc()
    nc = _NC_CACHE

    B = x.shape[0]
    in_maps = []
    for i in range(N_CORES):
        shard = x[i * B_PER_CORE : (i + 1) * B_PER_CORE]
        in_maps.append(prep_core_inputs(shard, w_conv, w_se1, w_se_out))

    global LAST_RESULT
    res = run_bass_kernel_spmd(nc, in_maps, core_ids=list(range(N_CORES)))
    LAST_RESULT = res
    out = np.concatenate(
        [postprocess(res.results[i]["out"]) for i in range(N_CORES)], axis=0
    )
    assert out.shape == (B, O, 56, 56)
    return out


# revision 9
# speedup vs baseline: 1.1981x; 1.1981x over previous
"""AdaptConv2d Trainium2 kernel: per-sample adapted 1x1 conv (SE-modulated).

Reference computation (B=32, C=O=256, H=W=56, HID=16):
    pooled = mean(x, (2,3))                      [B, C]
    hid    = silu(pooled @ w_se1.T)              [B, 16]
    adapt  = (hid @ w_se_out.T).reshape(B,O,C)   [B, O, C]
    out[b] = (w_conv + adapt[b]) @ x[b]          [B, O, H*W]

Distribution: data-parallel over batch, 4 samples per core on 8 cores, no
collectives. Per core everything is done on-chip:
  - x streamed in as bf16 (host pre-cast), kept SBUF-resident; junk
    matmuls on arriving tiles keep the PE HAM clock-gate warm
  - pooling via free-dim reduce/accumulate on ACT (+DVE for the last
    sample) while x loads
  - the SE hidden layer is computed directly in "block-diagonal" form:
    the stationary operand replicates w_se1 for 8 h-groups, the result
    z is turned into bd[(g,h),(g',b)] = silu(z[h,b]) * (g==g') with a
    constant mask, so the adapt matmul contracts all 128 partitions
  - adapt: single bf16 PE pass over w_se_out (8192 columns)
  - adapt is evicted to SBUF and reshaped into per-sample lhsT layout
    [C, O] via SBUF->SBUF DMAs, then w_conv is added
  - main GEMM: per sample [O,C]@[C,HW] as bf16 matmuls accumulating over
    C in PSUM, evicted to bf16 staging, batched 0.8MB output DMAs
"""

import numpy as np

B_PER_CORE = 4
N_CORES = 8
C = 256
O = 256
HW = 3136
HID = 16
P = 128
NT = 448  # free-dim tile of the main GEMM (7 * 448 = 3136)
NN = HW // NT
G = 8  # h-group packing factor for the adapt matmul
SEG = C * O // G // 256  # 32 c-rows per group block
ADN = C * O // G  # 8192 streaming columns for adapt
ADT = ADN // 512  # 16 psum tiles
JUNK = 5  # warmup matmuls per arriving x half-tile


def build_nc():
    from concourse import bacc, tile, mybir

    f32 = mybir.dt.float32
    bf16 = mybir.dt.bfloat16

    nc = bacc.Bacc("TRN2", target_bir_lowering=False, debug=False)

    x_d = nc.dram_tensor("x", [B_PER_CORE, P, 2 * HW], bf16, kind="ExternalInput")
    wse_d = nc.dram_tensor("w_se", [P, ADN], bf16, kind="ExternalInput")
    wconv_d = nc.dram_tensor("w_conv", [P, 2 * O], bf16, kind="ExternalInput")
    wse1_d = nc.dram_tensor("w_se1", [P, 2 * P], f32, kind="ExternalInput")
    mask_d = nc.dram_tensor("bd_mask", [P, G * B_PER_CORE], bf16, kind="ExternalInput")
    out_d = nc.dram_tensor("out", [B_PER_CORE, 2, P, HW], bf16, kind="ExternalOutput")

    with tile.TileContext(nc) as tc:
        with (
            tc.tile_pool(name="xp", bufs=B_PER_CORE) as xp,
            tc.tile_pool(name="wsep", bufs=1) as wsep,
            tc.tile_pool(name="consts", bufs=1) as consts,
            tc.tile_pool(name="scratch", bufs=2) as scratchp,
            tc.tile_pool(name="small", bufs=1) as small,
            tc.tile_pool(name="stage", bufs=2) as stagep,
            tc.tile_pool(name="ps_small", bufs=1, space="PSUM") as ps_small,
            tc.tile_pool(name="ps_mm", bufs=4, space="PSUM") as ps_mm,
        ):
            junk_ps = ps_mm.tile([P, 512], f32, tag="junk", bufs=1)

            # constants first (small, off the critical path)
            wconv_sb = consts.tile([P, 2 * O], bf16, tag="wconv")
            nc.sync.dma_start(out=wconv_sb[:], in_=wconv_d.ap()[:])
            wse1_sb = consts.tile([P, 2 * P], f32, tag="wse1")
            nc.sync.dma_start(out=wse1_sb[:], in_=wse1_d.ap()[:])
            mask_sb = consts.tile([P, G * B_PER_CORE], bf16, tag="mask")
            nc.sync.dma_start(out=mask_sb[:], in_=mask_d.ap()[:])

            # prepay the sigmoid LUT load while DMAs stream
            lutw = small.tile([P, 1], f32, tag="lutw")
            nc.scalar.activation(
                lutw[:], wse1_sb[:, 0:1], mybir.ActivationFunctionType.Sigmoid
            )

            pooled = [
                small.tile([P, B_PER_CORE], f32, tag=f"pooled{k}", name=f"pooled{k}")
                for k in range(2)
            ]

            # ---- x loads (per half-tile), junk warmup MMs, pooling ----
            x_tiles = []
            for b in range(B_PER_CORE):
                xt = xp.tile([P, 2 * HW], bf16, tag="x", name=f"xt{b}")
                x_tiles.append(xt)
            wse_sb = wsep.tile([P, ADN], bf16)
            for b in range(B_PER_CORE):
                for k in range(2):
                    nc.sync.dma_start(
                        out=x_tiles[b][:, k * HW : (k + 1) * HW],
                        in_=x_d.ap()[b][:, k * HW : (k + 1) * HW],
                    )
                    for j in range(JUNK):
                        nc.tensor.matmul(
                            junk_ps[:],
                            lhsT=x_tiles[b][:, k * HW : k * HW + P],
                            rhs=x_tiles[b][:, k * HW : k * HW + 512],
                            start=True,
                            stop=True,
                        )
                    # pooling: last sample split DVE/ACT, rest on ACT
                    if b == B_PER_CORE - 1 and k == 0:
                        nc.vector.tensor_reduce(
                            out=pooled[k][:, b : b + 1],
                            in_=x_tiles[b][:, k * HW : (k + 1) * HW],
                            axis=mybir.AxisListType.X,
                            op=mybir.AluOpType.add,
                        )
                    else:
                        sca = scratchp.tile(
                            [P, HW], bf16, tag="poolscratch", name=f"psc{b}{k}"
                        )
                        nc.scalar.activation(
                            out=sca[:],
                            in_=x_tiles[b][:, k * HW : (k + 1) * HW],
                            func=mybir.ActivationFunctionType.Copy,
                            accum_out=pooled[k][:, b : b + 1],
                        )
                if b == 1:
                    # w_se arrives mid-stream: needed right after pooling ends
                    nc.sync.dma_start(out=wse_sb[:], in_=wse_d.ap()[:])

            # ---- bd = silu(z) * mask, z[(g,h),(g',b)] = z[h,b] ----
            # pooled_rep[k]: pooled[k] replicated over the 8 g' column groups
            pooled_rep = []
            for k in range(2):
                pr = small.tile(
                    [P, G * B_PER_CORE], f32, tag=f"prep{k}", name=f"prep{k}"
                )
                pooled_rep.append(pr)
                for g in range(G):
                    nc.vector.tensor_copy(
                        out=pr[:, g * B_PER_CORE : (g + 1) * B_PER_CORE],
                        in_=pooled[k][:],
                    )
            z_ps = ps_small.tile([P, G * B_PER_CORE], f32, tag="zps", bufs=1)
            for k in range(2):
                nc.tensor.matmul(
                    z_ps[:],
                    lhsT=wse1_sb[:, k * P : (k + 1) * P],
                    rhs=pooled_rep[k][:],
                    start=(k == 0),
                    stop=(k == 1),
                )
            sig_sb = small.tile([P, G * B_PER_CORE], f32, tag="sig")
            nc.scalar.activation(
                sig_sb[:], z_ps[:], mybir.ActivationFunctionType.Sigmoid
            )
            zs_sb = small.tile([P, G * B_PER_CORE], f32, tag="zs")
            nc.vector.tensor_tensor(
                out=zs_sb[:], in0=sig_sb[:], in1=z_ps[:], op=mybir.AluOpType.mult
            )
            bd = small.tile([P, G * B_PER_CORE], bf16, tag="bd")
            nc.vector.tensor_tensor(
                out=bd[:], in0=zs_sb[:], in1=mask_sb[:], op=mybir.AluOpType.mult
            )

            # ---- adapt matmul: out[(g,b), n] = adapt[b, flat=g*8192+n] ----
            # (flat index = c*256 + o)
            adapt_sb = wsep.tile([G * B_PER_CORE, ADN], bf16, tag="adapt")
            for t in range(ADT):
                ap_ps = ps_small.tile(
                    [G * B_PER_CORE, 512], f32, tag="adps", name=f"adps{t}", bufs=2
                )
                nc.tensor.matmul(
                    ap_ps[:],
                    lhsT=bd[:],
                    rhs=wse_sb[:, t * 512 : (t + 1) * 512],
                    start=True,
                    stop=True,
                )
                if t % 2 == 0:
                    nc.vector.tensor_copy(
                        out=adapt_sb[:, t * 512 : (t + 1) * 512], in_=ap_ps[:]
                    )
                else:
                    nc.scalar.copy(
                        out=adapt_sb[:, t * 512 : (t + 1) * 512], in_=ap_ps[:]
                    )

            # ---- reshape adapt rows into lhsT layout + add w_conv ----
            # wb[:, (b*2+k)*256 + o] over partitions c_local = lhsT[c, o]
            wb_raw = consts.tile([P, B_PER_CORE * 2 * O], bf16, tag="wbraw")
            wb = consts.tile([P, B_PER_CORE * 2 * O], bf16, tag="wb")
            for b in range(B_PER_CORE):
                for k in range(2):
                    col0 = (b * 2 + k) * O
                    for gl in range(4):
                        g = k * 4 + gl
                        r = g * B_PER_CORE + b
                        eng = nc.sync if (gl % 2 == 0) else nc.scalar
                        eng.dma_start(
                            out=wb_raw[gl * SEG : (gl + 1) * SEG, col0 : col0 + O],
                            in_=adapt_sb[r : r + 1, :],
                        )
                    nc.vector.tensor_tensor(
                        out=wb[:, col0 : col0 + O],
                        in0=wb_raw[:, col0 : col0 + O],
                        in1=wconv_sb[:, k * O : (k + 1) * O],
                        op=mybir.AluOpType.add,
                    )

            # ---- main GEMM ----
            for b in range(B_PER_CORE):
                for oc in range(2):
                    stage = stagep.tile([P, HW], bf16, tag="stage", name=f"st{b}{oc}")
                    for n in range(NN):
                        ps = ps_mm.tile([P, NT], f32, tag="mmps", name=f"ps{b}{oc}{n}")
                        for k in range(2):
                            nc.tensor.matmul(
                                ps[:],
                                lhsT=wb[
                                    :,
                                    (b * 2 + k) * O
                                    + oc * P : (b * 2 + k) * O
                                    + oc * P
                                    + P,
                                ],
                                rhs=x_tiles[b][
                                    :, k * HW + n * NT : k * HW + (n + 1) * NT
                                ],
                                start=(k == 0),
                                stop=(k == 1),
                            )
                        if n % 2 == 0:
                            nc.vector.tensor_copy(
                                out=stage[:, n * NT : (n + 1) * NT], in_=ps[:]
                            )
                        else:
                            nc.scalar.copy(
                                out=stage[:, n * NT : (n + 1) * NT], in_=ps[:]
                            )
                    nc.scalar.dma_start(out=out_d.ap()[b, oc], in_=stage[:])

    nc.compile()
    return nc


def prep_core_inputs(x_shard, w_conv, w_se1, w_se_out):
    """Host-side layout prep for one core. x_shard: [4, 256, 56, 56] f32."""
    import ml_dtypes

    bf16 = ml_dtypes.bfloat16
    b = x_shard.shape[0]
    # x: [b, 128, 2*3136], c = k*128 + p, free = k*3136 + hw
    xr = x_shard.reshape(b, 2, P, HW).transpose(0, 2, 1, 3).reshape(b, P, 2 * HW)
    x_dev = np.ascontiguousarray(xr).astype(bf16)
    # w_se: [(g,h), n] with flat = c*256 + o = g*8192 + n
    w_r = w_se_out.reshape(O, C, HID).transpose(1, 0, 2)  # [c, o, h]
    w_r = w_r.reshape(G, ADN, HID).transpose(0, 2, 1).reshape(P, ADN)
    wse_dev = np.ascontiguousarray(w_r).astype(bf16)
    # w_conv: [p, k*256 + o] = w_conv[o, k*128+p]
    wc = w_conv[:, :, 0, 0].T.reshape(2, P, O).transpose(1, 0, 2).reshape(P, 2 * O)
    wconv_dev = np.ascontiguousarray(wc).astype(bf16)
    # w_se1 replicated for the G h-groups:
    # [p, k*128 + (g*16+h)] = w_se1[h, k*128+p] / 3136
    w1 = (w_se1.T / float(HW)).reshape(2, P, HID)  # [k, p, h]
    w1 = np.broadcast_to(w1[:, :, None, :], (2, P, G, HID)).reshape(2, P, P)
    w1 = np.ascontiguousarray(w1.transpose(1, 0, 2).reshape(P, 2 * P)).astype(
        np.float32
    )
    # bd mask: [(g,h), (g',b)] = 1 if g == g'
    m = np.zeros((G, HID, G, B_PER_CORE), np.float32)
    for g in range(G):
        m[g, :, g, :] = 1.0
    mask_dev = m.reshape(P, G * B_PER_CORE).astype(bf16)
    return {
        "x": x_dev,
        "w_se": wse_dev,
        "w_conv": wconv_dev,
        "w_se1": w1,
        "bd_mask": mask_dev,
    }


def postprocess(raw_out):
    """raw_out: [4, 2, 128, 3136] bf16 -> [4, 256, 56, 56] f32."""
    return np.asarray(raw_out, dtype=np.float32).reshape(B_PER_CORE, O, 56, 56)


_NC_CACHE = None
LAST_RESULT = None


def kernel(x, w_conv, w_se1, w_se_out):
    global _NC_CACHE
    from concourse.bass_utils import run_bass_kernel_spmd

    if _NC_CACHE is None:
        _NC_CACHE = build_nc()
    nc = _NC_CACHE

    B = x.shape[0]
    in_maps = []
    for i in range(N_CORES):
        shard = x[i * B_PER_CORE : (i + 1) * B_PER_CORE]
        in_maps.append(prep_core_inputs(shard, w_conv, w_se1, w_se_out))

    global LAST_RESULT
    res = run_bass_kernel_spmd(nc, in_maps, core_ids=list(range(N_CORES)))
    LAST_RESULT = res
    out = np.concatenate(
        [postprocess(res.results[i]["out"]) for i in range(N_CORES)], axis=0
    )
    assert out.shape == (B, O, 56, 56)
    return out
